# revision 19
# baseline (speedup 1.0000x reference)
"""GQA attention (B=2, S=2048, D=2048, 16 q-heads / 4 kv-heads, RoPE, causal)
for 8 Trainium2 NeuronCores.

Sharding: core c = 4*b + g handles batch b and GQA group g (q-heads 4g..4g+3,
kv-head g). Each core computes q/k/v projections for its group, RoPE, causal
attention, and the partial output projection attn @ wo[rows of its heads].
The host sums the 4 partials per batch (the only cross-core reduction).

Everything on device is bf16 (f32 PSUM accumulate); host pre-casts inputs.
The rel-err budget (2e-2) dwarfs bf16 noise (~4e-3 measured) and bf16 buys:
half the DMA bytes, full-rate matmuls at ANY free size (f32r needs >=256),
VN=129 PV matmuls, and 1 cycle/row PE transposes.

Host-side preprocessing folded into the inputs:
- xT = x[b].T so projections need no on-device transpose.
- wq/wk columns permuted per head from interleaved (even,odd) RoPE pairs to
  half-split ([evens | odds]) so RoPE becomes ops on contiguous 64-row halves.
- 1/sqrt(head_dim) folded into wq.
- 0/1 bf16 causal mask for the diagonal 128x128 block, [k, q] orientation
  (applied multiplicatively AFTER exp, into a separate ptd tile).

Device data flow (per core):
  phase 1 (s-slice outer, d-chunk inner; weight/x DMAs batched 4-8 chunks
  per dma_start — HWDGE descriptor-gen is a serial ~0.6us/DMA resource —
  and d-interleaved so the first matmul starts ~2us in): two sets of 3
  parallel PSUM accumulators (A = q0,q1,q2; B = q3,k,v) so set A's RoPE
  eviction overlaps set B's matmuls. RoPE per eviction = 4 half-width
  PSUM-sourced muls (DVE; walrus requires equal base partitions when both
  TensorTensor inputs are SBUF, so products land in base-0 tiles) + 2
  cross-half sub/add (Pool, SBUF-only — GPSIMD cannot touch PSUM). v is
  copied bf16 to SBUF (Act), PE-transposed into natural [k, hd] layout in
  v_all, whose col 128 is a memset ones column (softmax denominators ride
  the PV matmul for free). The first scores chunks of j=0,1 run at the tail
  of phase 1 on a spare PSUM bank so the merged phase starts hot.

  phases 2+3 (merged, software-pipelined per 128-row q/k block i):
    scores chunk (h, j, c) covers probsT[h][j] cols [512c, 512c+512) and is
    first needed by PV at step i = j + 4c; chunks are emitted two steps
    early, spread between PV blocks so the in-order PE queue always holds
    ready work and Act (exp) is fed at a steady rate. Per chunk: scoresT
    [k,q] = kT_j.T @ qT (PE) -> exp -> bf16 probsT (Act); diagonal chunks
    also write a mask-multiplied ptd tile (Pool). Per (i,h): PV accumulates
    probsT_j.T @ v_all_j[:, :129] (col 128 = denominator), reciprocal (DVE),
    normalize (Act, scale-ptr copy), PE-transpose, copy to attnT (DVE).
    Per i: out[i-block] = sum_h attnT_h.T @ wo_h, evicted bf16 (DVE) and
    DMA'd out; the host sums 4 bf16 partials per batch in f32.

Softmax skips max-subtraction: scores are ~N(0,1) by construction so exp()
cannot overflow in f32 (masked-out diagonal entries flow through exp and are
zeroed after).
"""

import numpy as np

import concourse.bass as bass
import concourse.mybir as mybir
import concourse.tile as tile
from concourse import bacc
from concourse.masks import make_identity

F32 = mybir.dt.float32
BF16 = mybir.dt.bfloat16

B = 2
S = 2048
D = 2048
N_HEADS = 16
N_KV_HEADS = 4
HD = 128  # head dim
HC = N_HEADS // N_KV_HEADS  # q-heads per core (= per kv group) = 4
N_CORES = 8
NEG = -1e30

PB = 128   # partition block
SB = 512   # matmul free-dim slice
VBLK = 132  # v_all per-k-block column stride (128 v cols + ones col + pad)
VN = 129    # PV matmul free dim (v cols + ones col)
ECHUNK = 1024  # scores exp chunk width (2 PSUM banks)

N_D = D // PB   # 16 contraction chunks
N_S = S // SB   # 4 column slices
N_KB = S // PB  # 16 k/q blocks


def emit_core_kernel(nc, tc, io, repeat=1):
    """Emit one core's program. io: dict of dram tensor handles."""
    xT, wq, wk, wv, wo = io["xT"], io["wq"], io["wk"], io["wv"], io["wo"]
    cosT, sinT, maskT, out = io["cosT"], io["sinT"], io["maskT"], io["out"]

    with tc.tile_pool(name="consts", bufs=1) as consts:
        ident = consts.tile([PB, PB], BF16, tag="ident")
        make_identity(nc, ident[:, :])
        mask_sb = consts.tile([PB, PB], BF16, tag="mask")
        nc.sync.dma_start(out=mask_sb[:, :], in_=maskT[:, :])

        for _rep in range(repeat):
            with (
                tc.tile_pool(name="qkv_out", bufs=1) as qkv_out,
                tc.tile_pool(name="early", bufs=1) as early,
            ):
                qT = qkv_out.tile([PB, HC * S], BF16, tag="qT")
                kT = qkv_out.tile([PB, S], BF16, tag="kT")
                v_all = qkv_out.tile([PB, N_KB * VBLK], BF16, tag="v")
                v_r = v_all[:, :].rearrange("p (j c) -> p j c", c=VBLK)
                nc.gpsimd.memset(v_r[:, :, HD:HD + 1], 1.0)

                # probsT state shared between the phase-1 prologue chunks and
                # the merged phase. pt tiles for j<2 plus the masked-diagonal
                # tiles live in `early` (open during phase 1); the rest go in
                # p2t, which opens after the phase-1 pools close.
                pts = {}
                ptds = {}

                def emit_chunk(h, j, c, tpool, pspool, pss_tag, pss_bufs, pw=ECHUNK):
                    # one scores+exp chunk of probsT[h][j], pw cols per psum
                    wj = S - j * PB
                    if (h, j) not in pts:
                        pts[(h, j)] = tpool.tile(
                            [PB, wj], BF16, tag=f"pt{h}_{j}", bufs=1,
                            name=f"pt{h}_{j}",
                        )
                    pt = pts[(h, j)]
                    c0 = c * ECHUNK
                    cw = min(ECHUNK, wj - c0)
                    for sub in range(0, cw, pw):
                        sw = min(pw, cw - sub)
                        q0 = j * PB + c0 + sub
                        pss = pspool.tile([PB, pw], F32, tag=pss_tag, bufs=pss_bufs)
                        for mm in range(0, sw, SB):
                            mw = min(SB, sw - mm)
                            nc.tensor.matmul(
                                pss[:, mm:mm + mw],
                                kT[:, j * PB:(j + 1) * PB],
                                qT[:, h * S + q0 + mm: h * S + q0 + mm + mw],
                                start=True, stop=True,
                            )
                        nc.scalar.activation(
                            pt[:, c0 + sub:c0 + sub + sw], pss[:, 0:sw],
                            mybir.ActivationFunctionType.Exp,
                        )
                    if c == 0:
                        # causal: masked diagonal block written to a separate
                        # tile (avoids RMW on pt; PV's j==i matmul reads ptd)
                        ptd = early.tile(
                            [PB, PB], BF16, tag=f"ptd{h}", bufs=2,
                            name=f"ptd{h}",
                        )
                        ptds[(h, j)] = ptd
                        nc.gpsimd.tensor_mul(
                            ptd[:, :], pt[:, 0:PB], mask_sb[:, :]
                        )

                # ============== phase 1: QKV projection + RoPE ==============
                with (
                    tc.tile_pool(name="w1", bufs=1) as w1,
                    tc.tile_pool(name="p1t", bufs=1) as p1t,
                    tc.tile_pool(name="p1ps", bufs=1, space="PSUM") as p1ps,
                    tc.tile_pool(name="p0ps", bufs=1, space="PSUM") as p0ps,
                ):
                    def early_chunk(h, j):
                        emit_chunk(h, j, 0, early, p0ps, "pss0", 1, pw=SB)
                    cs = w1.tile([PB, S], F32, tag="cs")
                    csw = w1.tile([PB, S], F32, tag="csw")
                    wq_sb = w1.tile([PB, N_D * HC * HD], BF16, tag="wq")
                    wk_sb = w1.tile([PB, N_D * HD], BF16, tag="wk")
                    wv_sb = w1.tile([PB, N_D * HD], BF16, tag="wv")
                    xts = {}

                    def load_xt(s, g):
                        # one DMA = 4 d-chunks of x for slice s
                        xt4 = p1t.tile([PB, 4 * SB], BF16, tag="xt", bufs=8, name=f"xt{s}_{g}")
                        nc.sync.dma_start(
                            out=xt4[:, :].rearrange("p (t f) -> p t f", f=SB),
                            in_=xT[g * 4 * PB:(g + 1) * 4 * PB, s * SB:(s + 1) * SB]
                            .rearrange("(t p) f -> p t f", p=PB),
                        )
                        for dd in range(4):
                            xts[(s, g * 4 + dd)] = xt4[:, dd * SB:(dd + 1) * SB]

                    # batched, d-interleaved weight + x loads (few DMA instrs;
                    # HWDGE descriptor-gen is a serial ~0.6us/DMA resource).
                    # First d-chunk of wq + first x tile go alone so the first
                    # matmul starts ~2us in; cos/sin (needed ~13us in) follow
                    # the first two groups.
                    def load_wq(g):
                        d0, d1 = (1, 4) if g == 0 else (4 * g, 4 * g + 4)
                        nc.sync.dma_start(
                            out=wq_sb[:, d0 * HC * HD:d1 * HC * HD]
                            .rearrange("p (t f) -> p t f", f=HC * HD),
                            in_=wq[d0 * PB:d1 * PB, :]
                            .rearrange("(t p) f -> p t f", p=PB),
                        )

                    def load_wkv(w_dram, w_sb, g):
                        nc.sync.dma_start(
                            out=w_sb[:, g * 8 * HD:(g + 1) * 8 * HD]
                            .rearrange("p (t f) -> p t f", f=HD),
                            in_=w_dram[g * 8 * PB:(g + 1) * 8 * PB, :]
                            .rearrange("(t p) f -> p t f", p=PB),
                        )

                    nc.sync.dma_start(out=wq_sb[:, 0:HC * HD], in_=wq[0:PB, :])
                    load_xt(0, 0)
                    load_wq(0)
                    load_xt(0, 1)
                    load_wq(1)
                    load_wkv(wk, wk_sb, 0)
                    nc.sync.dma_start(out=cs[0:64, :], in_=cosT[:, :])
                    nc.sync.dma_start(out=cs[64:128, :], in_=sinT[:, :])
                    load_xt(0, 2)
                    load_wq(2)
                    load_wkv(wv, wv_sb, 0)
                    nc.sync.dma_start(out=csw[0:64, :], in_=sinT[:, :])
                    nc.sync.dma_start(out=csw[64:128, :], in_=cosT[:, :])
                    load_xt(0, 3)
                    load_wq(3)
                    load_wkv(wk, wk_sb, 1)
                    load_wkv(wv, wv_sb, 1)
                    load_xt(1, 0)

                    def rope_evict(ps, s, dest_r, dest_i):
                        # ps rows 0:64 even half (r), 64:128 odd half (i)
                        # out_r = r*cos - i*sin ; out_i = r*sin + i*cos
                        # Muls (PSUM-source) on DVE into base-0 tiles (walrus
                        # requires equal base partitions when both TT inputs
                        # are SBUF); cross-half sub/add on Pool.
                        sl = slice(s * SB, (s + 1) * SB)
                        t1 = p1t.tile([64, SB], BF16, tag="t1", bufs=3)
                        t2 = p1t.tile([64, SB], BF16, tag="t2", bufs=3)
                        t3 = p1t.tile([64, SB], BF16, tag="t3", bufs=3)
                        t4 = p1t.tile([64, SB], BF16, tag="t4", bufs=3)
                        nc.vector.tensor_mul(t1[:, :], ps[0:64, 0:SB], cs[0:64, sl])
                        nc.vector.tensor_mul(t2[:, :], ps[64:128, 0:SB], cs[64:128, sl])
                        nc.vector.tensor_mul(t3[:, :], ps[0:64, 0:SB], csw[0:64, sl])
                        nc.vector.tensor_mul(t4[:, :], ps[64:128, 0:SB], csw[64:128, sl])
                        nc.gpsimd.tensor_sub(dest_r, t1[:, :], t2[:, :])
                        nc.gpsimd.tensor_add(dest_i, t3[:, :], t4[:, :])

                    for s in range(N_S):
                        if s > 0:
                            for g in range(4):
                                if (s, g * 4) not in xts:
                                    load_xt(s, g)
                        # set A: q heads 0..2 — evictions overlap set B matmuls
                        psA = [p1ps.tile([PB, SB], F32, tag="proj", bufs=6, name=f"psA{t}") for t in range(3)]
                        for d in range(N_D):
                            for t in range(3):
                                nc.tensor.matmul(
                                    psA[t][:, :],
                                    wq_sb[:, d * HC * HD + t * HD: d * HC * HD + (t + 1) * HD],
                                    xts[(s, d)][:, :],
                                    start=(d == 0),
                                    stop=(d == N_D - 1),
                                )
                        if s + 1 < N_S:
                            load_xt(s + 1, 0)
                            load_xt(s + 1, 1)
                        for t in range(3):
                            rope_evict(
                                psA[t], s,
                                qT[0:64, t * S + s * SB: t * S + (s + 1) * SB],
                                qT[64:128, t * S + s * SB: t * S + (s + 1) * SB],
                            )
                        if s == 2:
                            early_chunk(0, 0)
                            early_chunk(1, 0)
                        elif s == 3:
                            early_chunk(0, 1)
                            early_chunk(1, 1)
                        # set B: q3, k, v
                        psB = [p1ps.tile([PB, SB], F32, tag="proj", bufs=6, name=f"psB{t}") for t in range(3)]
                        for d in range(N_D):
                            nc.tensor.matmul(
                                psB[0][:, :],
                                wq_sb[:, d * HC * HD + 3 * HD: d * HC * HD + 4 * HD],
                                xts[(s, d)][:, :],
                                start=(d == 0), stop=(d == N_D - 1),
                            )
                            nc.tensor.matmul(
                                psB[1][:, :],
                                wk_sb[:, d * HD:(d + 1) * HD],
                                xts[(s, d)][:, :],
                                start=(d == 0), stop=(d == N_D - 1),
                            )
                            nc.tensor.matmul(
                                psB[2][:, :],
                                wv_sb[:, d * HD:(d + 1) * HD],
                                xts[(s, d)][:, :],
                                start=(d == 0), stop=(d == N_D - 1),
                            )
                        b_evicts = [
                            (psB[0],
                             qT[0:64, 3 * S + s * SB: 3 * S + (s + 1) * SB],
                             qT[64:128, 3 * S + s * SB: 3 * S + (s + 1) * SB]),
                            (psB[1],
                             kT[0:64, s * SB:(s + 1) * SB],
                             kT[64:128, s * SB:(s + 1) * SB]),
                        ]
                        if s == N_S - 1:
                            b_evicts.reverse()
                        for ps_, dr, di in b_evicts:
                            rope_evict(ps_, s, dr, di)
                        # v: bf16 copy to SBUF, PE-transpose 128-blocks into
                        # natural [k, hd] layout
                        vt = p1t.tile([PB, SB], BF16, tag="vt", bufs=2)
                        nc.scalar.copy(vt[:, :], psB[2][:, :])
                        for sb_i in range(SB // PB):
                            j = s * (SB // PB) + sb_i
                            pstv = p1ps.tile([PB, PB], BF16, tag="pstv", bufs=1)
                            nc.tensor.transpose(
                                pstv[:, :], vt[:, sb_i * PB:(sb_i + 1) * PB], ident[:, :]
                            )
                            nc.scalar.copy(
                                v_all[:, j * VBLK: j * VBLK + HD], pstv[:, :]
                            )
                        if s == 2:
                            early_chunk(2, 0)
                            early_chunk(3, 0)
                        elif s == 3:
                            early_chunk(2, 1)
                            early_chunk(3, 1)

                # ============== phases 2+3 (merged, pipelined) ==============
                with tc.tile_pool(name="attp", bufs=1) as attp:
                    wo_sb = attp.tile([PB, HC * D], BF16, tag="wo")
                    for h in range(HC):
                        nc.sync.dma_start(
                            out=wo_sb[:, h * D:(h + 1) * D],
                            in_=wo[h * PB:(h + 1) * PB, :],
                        )
                    with (
                        tc.tile_pool(name="p2t", bufs=1) as p2t,
                        tc.tile_pool(name="p2ps", bufs=1, space="PSUM") as p2ps,
                    ):
                        atts = {}

                        def emit_chunk2(h, j, c):
                            emit_chunk(h, j, c, p2t, p2ps, "pss", 2)

                        def chunks_for(i):
                            # lazy: chunk (h, j, c) is first needed by PV(i)
                            # at i = j + 8*c  (chunk covers 8 q-blocks)
                            cl = []
                            for c in range(2):
                                j = i - 8 * c
                                if 0 <= j < N_KB and c * ECHUNK < S - j * PB:
                                    cl.extend((h, j, c) for h in range(HC))
                            return cl

                        def emit_pv1(i, h):
                            psa = p2ps.tile([PB, VN], F32, tag="psa", bufs=2)
                            for j in range(i + 1):
                                lhsT = (
                                    ptds[(h, j)][:, :] if j == i
                                    else pts[(h, j)][:, (i - j) * PB:(i - j + 1) * PB]
                                )
                                nc.tensor.matmul(
                                    psa[:, :],
                                    lhsT,
                                    v_all[:, j * VBLK: j * VBLK + VN],
                                    start=(j == 0), stop=(j == i),
                                )
                            rinv = p2t.tile([PB, 1], F32, tag="rinv", bufs=4)
                            nc.vector.reciprocal(rinv[:, :], psa[:, HD:HD + 1])
                            attn = p2t.tile([PB, PB], BF16, tag="attn", bufs=4)
                            nc.scalar.mul(attn[:, :], psa[:, 0:HD], rinv[:, :])
                            pstT = p2ps.tile([PB, PB], BF16, tag="pstT", bufs=1)
                            nc.tensor.transpose(pstT[:, :], attn[:, :], ident[:, :])
                            at = p2t.tile([PB, PB], BF16, tag="attnT", bufs=12)
                            nc.vector.tensor_copy(at[:, :], pstT[:, :])
                            atts[(i, h)] = at

                        def emit_p3(i, split_dma=False):
                            ot = p2t.tile([PB, D], BF16, tag="ot", bufs=2)
                            for n0 in range(0, D, SB):
                                pso = p2ps.tile([PB, SB], F32, tag="pso", bufs=1)
                                for h in range(HC):
                                    nc.tensor.matmul(
                                        pso[:, :],
                                        atts[(i, h)][:, :],
                                        wo_sb[:, h * D + n0: h * D + n0 + SB],
                                        start=(h == 0), stop=(h == HC - 1),
                                    )
                                nc.vector.tensor_copy(ot[:, n0:n0 + SB], pso[:, :])
                                if split_dma:
                                    nc.sync.dma_start(
                                        out=out[i * PB:(i + 1) * PB, n0:n0 + SB],
                                        in_=ot[:, n0:n0 + SB],
                                    )
                            if not split_dma:
                                nc.sync.dma_start(out=out[i * PB:(i + 1) * PB, :], in_=ot[:, :])

                        # software pipeline: all chunks PV(i) reads are emitted
                        # before any PV(i, h); the NEXT step's chunks spread
                        # between this step's PV/p3 work so the in-order PE
                        # queue always holds dependency-ready work and the Act
                        # engine is fed at a steady rate.
                        for t in chunks_for(0) + chunks_for(1):
                            if (t[0], t[1]) in pts and t[2] == 0:
                                continue  # emitted during phase 1
                            emit_chunk2(*t)
                        for i in range(N_KB):
                            nxt = chunks_for(i + 2) if i + 2 < N_KB else []
                            nn = max(1, len(nxt))
                            emit_pv1(i, 0)
                            for t in nxt[: nn // 3]:
                                emit_chunk2(*t)
                            emit_pv1(i, 1)
                            if i >= 1:
                                emit_p3(i - 1)
                            for t in nxt[nn // 3: 2 * nn // 3]:
                                emit_chunk2(*t)
                            emit_pv1(i, 2)
                            for t in nxt[2 * nn // 3:]:
                                emit_chunk2(*t)
                            emit_pv1(i, 3)
                        emit_p3(N_KB - 1, split_dma=True)


def build_nc(repeat=1):
    nc = bacc.Bacc("TRN2", target_bir_lowering=False, debug=False, num_devices=N_CORES)
    io = {
        "xT": nc.dram_tensor("xT", [D, S], BF16, kind="ExternalInput"),
        "wq": nc.dram_tensor("wq", [D, HC * HD], BF16, kind="ExternalInput"),
        "wk": nc.dram_tensor("wk", [D, HD], BF16, kind="ExternalInput"),
        "wv": nc.dram_tensor("wv", [D, HD], BF16, kind="ExternalInput"),
        "wo": nc.dram_tensor("wo", [HC * HD, D], BF16, kind="ExternalInput"),
        "cosT": nc.dram_tensor("cosT", [HD // 2, S], F32, kind="ExternalInput"),
        "maskT": nc.dram_tensor("maskT", [PB, PB], BF16, kind="ExternalInput"),
        "sinT": nc.dram_tensor("sinT", [HD // 2, S], F32, kind="ExternalInput"),
        "out": nc.dram_tensor("out", [S, D], BF16, kind="ExternalOutput"),
    }
    with tile.TileContext(nc) as tc:
        emit_core_kernel(nc, tc, io, repeat=repeat)
    nc.compile()
    return nc


# ---------------------------------------------------------------------------
# host-side sharding + execution
# ---------------------------------------------------------------------------

_HALFSPLIT = np.concatenate([np.arange(0, HD, 2), np.arange(1, HD, 2)])


def _bf16():
    import ml_dtypes
    return ml_dtypes.bfloat16


def make_core_inputs(x, wq, wk, wv, wo, freqs_cos, freqs_sin):
    """Build the 8 per-core input dicts (numpy, host-side)."""
    bf16 = _bf16()
    scale = np.float32(1.0 / np.sqrt(HD))

    maskT = np.where(
        np.arange(PB)[None, :] >= np.arange(PB)[:, None], 1.0, 0.0
    ).astype(bf16)  # [k, q]: keep where q >= k
    xTs = [np.ascontiguousarray(x[b].T).astype(bf16) for b in range(B)]
    cosTs = [np.ascontiguousarray(freqs_cos[b].T).astype(np.float32) for b in range(B)]
    sinTs = [np.ascontiguousarray(freqs_sin[b].T).astype(np.float32) for b in range(B)]

    in_maps = []
    for c in range(N_CORES):
        b, g = divmod(c, N_KV_HEADS)
        qcols = np.concatenate([(HC * g + h) * HD + _HALFSPLIT for h in range(HC)])
        wq_c = (np.ascontiguousarray(wq[:, qcols]) * scale).astype(bf16)
        wk_c = np.ascontiguousarray(wk[:, g * HD + _HALFSPLIT]).astype(bf16)
        wv_c = np.ascontiguousarray(wv[:, g * HD:(g + 1) * HD]).astype(bf16)
        wo_c = np.ascontiguousarray(wo[g * HC * HD:(g + 1) * HC * HD, :]).astype(bf16)
        in_maps.append(
            {
                "xT": xTs[b],
                "wq": wq_c,
                "wk": wk_c,
                "wv": wv_c,
                "wo": wo_c,
                "cosT": cosTs[b],
                "maskT": maskT,
                "sinT": sinTs[b],
            }
        )
    return in_maps


_CACHE = {}


def get_runner(repeat=1, chain=1):
    """Build (once) the Bass module and a cached jitted 8-core executor."""
    if (repeat, chain) in _CACHE:
        return _CACHE[(repeat, chain)]
    import jax
    from jax.sharding import Mesh, PartitionSpec
    from jax.experimental.shard_map import shard_map
    from concourse.bass2jax import (
        _bass_exec_p,
        install_neuronx_cc_hook,
        partition_id_tensor,
    )

    nc = build_nc(repeat=repeat)
    install_neuronx_cc_hook()
    partition_name = nc.partition_id_tensor.name if nc.partition_id_tensor else None
    in_names, out_names, out_avals = [], [], []
    for alloc in nc.m.functions[0].allocations:
        if not isinstance(alloc, mybir.MemoryLocationSet):
            continue
        name = alloc.memorylocations[0].name
        if alloc.kind == "ExternalInput":
            if name != partition_name:
                in_names.append(name)
        elif alloc.kind == "ExternalOutput":
            out_names.append(name)
            out_avals.append(
                jax.core.ShapedArray(tuple(alloc.tensor_shape), mybir.dt.np(alloc.dtype))
            )
    n_params = len(in_names)
    n_outs = len(out_avals)
    all_in_names = list(in_names) + list(out_names)
    if partition_name is not None:
        all_in_names.append(partition_name)

    def _body(*args):
        operands = list(args)
        if partition_name is not None:
            operands.append(partition_id_tensor())
        outs = _bass_exec_p.bind(
            *operands,
            out_avals=tuple(out_avals),
            in_names=tuple(all_in_names),
            out_names=tuple(out_names),
            lowering_input_output_aliases=(),
            sim_require_finite=True,
            sim_require_nnan=True,
            nc=nc,
        )
        return tuple(outs)

    devices = jax.devices()[:N_CORES]
    mesh = Mesh(np.asarray(devices), ("core",))
    in_specs = (PartitionSpec("core"),) * (n_params + n_outs)
    out_specs = (PartitionSpec("core"),) * n_outs

    def _chain(*args):
        ins, outs = args[:n_params], args[n_params:]
        for _ in range(chain):
            outs = _body(*ins, *outs)
        return outs

    fn = jax.jit(
        shard_map(_chain, mesh=mesh, in_specs=in_specs, out_specs=out_specs, check_rep=False),
        keep_unused=True,
    )

    from jax.sharding import NamedSharding

    sh = NamedSharding(mesh, PartitionSpec("core"))

    def prepare(in_maps):
        concat_in = [
            np.concatenate([m[name] for m in in_maps], axis=0) for name in in_names
        ]
        concat_zeros = [
            np.zeros((N_CORES * a.shape[0], *a.shape[1:]), a.dtype) for a in out_avals
        ]
        return [jax.device_put(a, sh) for a in concat_in + concat_zeros]

    def run_dev(dev_args):
        out_arrs = fn(*dev_args)
        jax.block_until_ready(out_arrs)
        return out_arrs

    def run(in_maps):
        out_arrs = run_dev(prepare(in_maps))
        return np.asarray(out_arrs[0]).reshape(N_CORES, S, D)

    run.prepare = prepare
    run.run_dev = run_dev
    run.fn = fn
    _CACHE[(repeat, chain)] = run
    return run


def kernel(x, wq, wk, wv, wo, freqs_cos, freqs_sin):
    x = np.asarray(x, np.float32)
    wq = np.asarray(wq, np.float32)
    wk = np.asarray(wk, np.float32)
    wv = np.asarray(wv, np.float32)
    wo = np.asarray(wo, np.float32)
    freqs_cos = np.asarray(freqs_cos, np.float32)
    freqs_sin = np.asarray(freqs_sin, np.float32)

    in_maps = make_core_inputs(x, wq, wk, wv, wo, freqs_cos, freqs_sin)
    run = get_runner(repeat=1)
    partials = run(in_maps)  # [8, S, D] bf16
    partials = partials.astype(np.float32)
    out = np.stack(
        [partials[b * N_KV_HEADS:(b + 1) * N_KV_HEADS].sum(axis=0) for b in range(B)]
    )
    return out.astype(np.float32)


# revision 20
# speedup vs baseline: 1.1569x; 1.1569x over previous
"""GQA attention (B=2, S=2048, D=2048, 16 q-heads / 4 kv-heads, RoPE, causal)
for 8 Trainium2 NeuronCores.

Sharding: core c = 4*b + g handles batch b and GQA group g (q-heads 4g..4g+3,
kv-head g). Each core computes q/k/v projections for its group, RoPE, causal
attention, and the partial output projection attn @ wo[rows of its heads].
The host sums the 4 partials per batch (the only cross-core reduction).

Everything on device is bf16 (f32 PSUM accumulate); host pre-casts inputs.
The rel-err budget (2e-2) dwarfs bf16 noise (~4e-3 measured) and bf16 buys:
half the DMA bytes, full-rate matmuls at ANY free size (f32r needs >=256),
VN=129 PV matmuls, and 1 cycle/row PE transposes.

Host-side preprocessing folded into the inputs:
- xT = x[b].T so projections need no on-device transpose.
- wq/wk columns permuted per head from interleaved (even,odd) RoPE pairs to
  half-split ([evens | odds]) so RoPE becomes ops on contiguous 64-row halves.
- 1/sqrt(head_dim) folded into wq.
- 0/1 bf16 causal mask for the diagonal 128x128 block, [k, q] orientation
  (applied multiplicatively AFTER exp, into a separate ptd tile).

Device data flow (per core):
  phase 1 (s-slice outer, d-chunk inner; weight/x DMAs batched 4-8 chunks
  per dma_start — HWDGE descriptor-gen is a serial ~0.6us/DMA resource —
  and d-interleaved so the first matmul starts ~2us in): two sets of 3
  parallel PSUM accumulators (A = q0,q1,q2; B = q3,k,v) so set A's RoPE
  eviction overlaps set B's matmuls. RoPE per eviction = 4 half-width
  PSUM-sourced muls (DVE; walrus requires equal base partitions when both
  TensorTensor inputs are SBUF, so products land in base-0 tiles) + 2
  cross-half sub/add (Pool, SBUF-only — GPSIMD cannot touch PSUM). v is
  copied bf16 to SBUF (Act), PE-transposed into natural [k, hd] layout in
  v_all, whose col 128 is a memset ones column (softmax denominators ride
  the PV matmul for free). The first scores chunks of j=0,1 run at the tail
  of phase 1 on a spare PSUM bank so the merged phase starts hot.

  phases 2+3 (merged, software-pipelined per 128-row q/k block i):
    scores chunk (h, j, c) covers probsT[h][j] cols [512c, 512c+512) and is
    first needed by PV at step i = j + 4c; chunks are emitted two steps
    early, spread between PV blocks so the in-order PE queue always holds
    ready work and Act (exp) is fed at a steady rate. Per chunk: scoresT
    [k,q] = kT_j.T @ qT (PE) -> exp -> bf16 probsT (Act); diagonal chunks
    also write a mask-multiplied ptd tile (Pool). Per (i,h): PV accumulates
    probsT_j.T @ v_all_j[:, :129] (col 128 = denominator), reciprocal (DVE),
    normalize (Act, scale-ptr copy), PE-transpose, copy to attnT (DVE).
    Per i: out[i-block] = sum_h attnT_h.T @ wo_h, evicted bf16 (DVE) and
    DMA'd out; the host sums 4 bf16 partials per batch in f32.

Softmax skips max-subtraction: scores are ~N(0,1) by construction so exp()
cannot overflow in f32 (masked-out diagonal entries flow through exp and are
zeroed after).
"""

import numpy as np

import concourse.bass as bass
import concourse.mybir as mybir
import concourse.tile as tile
from concourse import bacc
from concourse.masks import make_identity

F32 = mybir.dt.float32
BF16 = mybir.dt.bfloat16

B = 2
S = 2048
D = 2048
N_HEADS = 16
N_KV_HEADS = 4
HD = 128  # head dim
HC = N_HEADS // N_KV_HEADS  # q-heads per core (= per kv group) = 4
N_CORES = 8
NEG = -1e30

PB = 128   # partition block
SB = 512   # matmul free-dim slice
VBLK = 132  # v_all per-k-block column stride (128 v cols + ones col + pad)
VN = 129    # PV matmul free dim (v cols + ones col)
ECHUNK = 512   # scores psum tile width (1 PSUM bank) = exp chunk

N_D = D // PB   # 16 contraction chunks
N_S = S // SB   # 4 column slices
N_KB = S // PB  # 16 k/q blocks


def emit_core_kernel(nc, tc, io, repeat=1):
    """Emit one core's program. io: dict of dram tensor handles."""
    xT, wq, wk, wv, wo = io["xT"], io["wq"], io["wk"], io["wv"], io["wo"]
    cosT, sinT, maskT, out = io["cosT"], io["sinT"], io["maskT"], io["out"]

    with tc.tile_pool(name="consts", bufs=1) as consts:
        ident = consts.tile([PB, PB], BF16, tag="ident")
        make_identity(nc, ident[:, :])
        mask_sb = consts.tile([PB, PB], BF16, tag="mask")
        nc.sync.dma_start(out=mask_sb[:, :], in_=maskT[:, :])

        for _rep in range(repeat):
            with (
                tc.tile_pool(name="qkv_out", bufs=1) as qkv_out,
                tc.tile_pool(name="early", bufs=1) as early,
            ):
                qT = qkv_out.tile([PB, HC * S], BF16, tag="qT")
                kT = qkv_out.tile([PB, S], BF16, tag="kT")
                v_all = qkv_out.tile([PB, N_KB * VBLK], BF16, tag="v")
                v_r = v_all[:, :].rearrange("p (j c) -> p j c", c=VBLK)
                nc.gpsimd.memset(v_r[:, :, HD:HD + 1], 1.0)

                # probsT state shared between the phase-1 prologue chunks and
                # the merged phase. pt tiles for j<2 plus the masked-diagonal
                # tiles live in `early` (open during phase 1); the rest go in
                # p2t, which opens after the phase-1 pools close.
                pts = {}
                ptds = {}

                def emit_chunk(h, j, c, tpool, pspool, pss_tag, pss_bufs):
                    # one 512-col scores+exp chunk of probsT[h][j]
                    wj = S - j * PB
                    if (h, j) not in pts:
                        pts[(h, j)] = tpool.tile(
                            [PB, wj], BF16, tag=f"pt{h}_{j}", bufs=1,
                            name=f"pt{h}_{j}",
                        )
                    pt = pts[(h, j)]
                    c0 = c * ECHUNK
                    cw = min(ECHUNK, wj - c0)
                    q0 = j * PB + c0
                    pss = pspool.tile([PB, ECHUNK], F32, tag=pss_tag, bufs=pss_bufs)
                    nc.tensor.matmul(
                        pss[:, 0:cw],
                        kT[:, j * PB:(j + 1) * PB],
                        qT[:, h * S + q0: h * S + q0 + cw],
                        start=True, stop=True,
                    )
                    nc.scalar.activation(
                        pt[:, c0:c0 + cw], pss[:, 0:cw],
                        mybir.ActivationFunctionType.Exp,
                    )
                    if c == 0:
                        # causal: masked diagonal block written to a separate
                        # tile (avoids RMW on pt; PV's j==i matmul reads ptd)
                        ptd = early.tile(
                            [PB, PB], BF16, tag=f"ptd{h}", bufs=2,
                            name=f"ptd{h}",
                        )
                        ptds[(h, j)] = ptd
                        nc.gpsimd.tensor_mul(
                            ptd[:, :], pt[:, 0:PB], mask_sb[:, :]
                        )

                # ============== phase 1: QKV projection + RoPE ==============
                with (
                    tc.tile_pool(name="w1", bufs=1) as w1,
                    tc.tile_pool(name="p1t", bufs=1) as p1t,
                    tc.tile_pool(name="p1ps", bufs=1, space="PSUM") as p1ps,
                    tc.tile_pool(name="p0ps", bufs=1, space="PSUM") as p0ps,
                ):
                    def early_chunk(h, j):
                        emit_chunk(h, j, 0, early, p0ps, "pss0", 1)
                    cs = w1.tile([PB, S], F32, tag="cs")
                    csw = w1.tile([PB, S], F32, tag="csw")
                    wq_sb = w1.tile([PB, N_D * HC * HD], BF16, tag="wq")
                    wk_sb = w1.tile([PB, N_D * HD], BF16, tag="wk")
                    wv_sb = w1.tile([PB, N_D * HD], BF16, tag="wv")
                    xts = {}

                    def load_xt(s, g):
                        # one DMA = 4 d-chunks of x for slice s
                        xt4 = p1t.tile([PB, 4 * SB], BF16, tag="xt", bufs=8, name=f"xt{s}_{g}")
                        nc.sync.dma_start(
                            out=xt4[:, :].rearrange("p (t f) -> p t f", f=SB),
                            in_=xT[g * 4 * PB:(g + 1) * 4 * PB, s * SB:(s + 1) * SB]
                            .rearrange("(t p) f -> p t f", p=PB),
                        )
                        for dd in range(4):
                            xts[(s, g * 4 + dd)] = xt4[:, dd * SB:(dd + 1) * SB]

                    # batched, d-interleaved weight + x loads (few DMA instrs;
                    # HWDGE descriptor-gen is a serial ~0.6us/DMA resource).
                    # First d-chunk of wq + first x tile go alone so the first
                    # matmul starts ~2us in; cos/sin (needed ~13us in) follow
                    # the first two groups.
                    def load_wq(g):
                        d0, d1 = (1, 4) if g == 0 else (4 * g, 4 * g + 4)
                        nc.sync.dma_start(
                            out=wq_sb[:, d0 * HC * HD:d1 * HC * HD]
                            .rearrange("p (t f) -> p t f", f=HC * HD),
                            in_=wq[d0 * PB:d1 * PB, :]
                            .rearrange("(t p) f -> p t f", p=PB),
                        )

                    def load_wkv(w_dram, w_sb, g):
                        nc.sync.dma_start(
                            out=w_sb[:, g * 8 * HD:(g + 1) * 8 * HD]
                            .rearrange("p (t f) -> p t f", f=HD),
                            in_=w_dram[g * 8 * PB:(g + 1) * 8 * PB, :]
                            .rearrange("(t p) f -> p t f", p=PB),
                        )

                    nc.sync.dma_start(out=wq_sb[:, 0:HC * HD], in_=wq[0:PB, :])
                    load_xt(0, 0)
                    load_wq(0)
                    load_xt(0, 1)
                    load_wq(1)
                    load_wkv(wk, wk_sb, 0)
                    nc.sync.dma_start(out=cs[0:64, :], in_=cosT[:, :])
                    nc.sync.dma_start(out=cs[64:128, :], in_=sinT[:, :])
                    load_xt(0, 2)
                    load_wq(2)
                    load_wkv(wv, wv_sb, 0)
                    nc.sync.dma_start(out=csw[0:64, :], in_=sinT[:, :])
                    nc.sync.dma_start(out=csw[64:128, :], in_=cosT[:, :])
                    load_xt(0, 3)
                    load_wq(3)
                    load_wkv(wk, wk_sb, 1)
                    load_wkv(wv, wv_sb, 1)
                    load_xt(1, 0)

                    def rope_evict(ps, s, dest_r, dest_i):
                        # ps rows 0:64 even half (r), 64:128 odd half (i)
                        # out_r = r*cos - i*sin ; out_i = r*sin + i*cos
                        # Muls (PSUM-source) on DVE into base-0 tiles (walrus
                        # requires equal base partitions when both TT inputs
                        # are SBUF); cross-half sub/add on Pool.
                        sl = slice(s * SB, (s + 1) * SB)
                        t1 = p1t.tile([64, SB], BF16, tag="t1", bufs=3)
                        t2 = p1t.tile([64, SB], BF16, tag="t2", bufs=3)
                        t3 = p1t.tile([64, SB], BF16, tag="t3", bufs=3)
                        t4 = p1t.tile([64, SB], BF16, tag="t4", bufs=3)
                        nc.vector.tensor_mul(t1[:, :], ps[0:64, 0:SB], cs[0:64, sl])
                        nc.vector.tensor_mul(t2[:, :], ps[64:128, 0:SB], cs[64:128, sl])
                        nc.vector.tensor_mul(t3[:, :], ps[0:64, 0:SB], csw[0:64, sl])
                        nc.vector.tensor_mul(t4[:, :], ps[64:128, 0:SB], csw[64:128, sl])
                        nc.gpsimd.tensor_sub(dest_r, t1[:, :], t2[:, :])
                        nc.gpsimd.tensor_add(dest_i, t3[:, :], t4[:, :])

                    for s in range(N_S):
                        if s > 0:
                            for g in range(4):
                                if (s, g * 4) not in xts:
                                    load_xt(s, g)
                        # set A: q heads 0..2 — evictions overlap set B matmuls
                        psA = [p1ps.tile([PB, SB], F32, tag="proj", bufs=6, name=f"psA{t}") for t in range(3)]
                        for d in range(N_D):
                            for t in range(3):
                                nc.tensor.matmul(
                                    psA[t][:, :],
                                    wq_sb[:, d * HC * HD + t * HD: d * HC * HD + (t + 1) * HD],
                                    xts[(s, d)][:, :],
                                    start=(d == 0),
                                    stop=(d == N_D - 1),
                                )
                        if s + 1 < N_S:
                            load_xt(s + 1, 0)
                            load_xt(s + 1, 1)
                        for t in range(3):
                            rope_evict(
                                psA[t], s,
                                qT[0:64, t * S + s * SB: t * S + (s + 1) * SB],
                                qT[64:128, t * S + s * SB: t * S + (s + 1) * SB],
                            )
                        if s == 2:
                            early_chunk(0, 0)
                            early_chunk(1, 0)
                        elif s == 3:
                            early_chunk(0, 1)
                            early_chunk(1, 1)
                        # set B: q3, k, v
                        psB = [p1ps.tile([PB, SB], F32, tag="proj", bufs=6, name=f"psB{t}") for t in range(3)]
                        for d in range(N_D):
                            nc.tensor.matmul(
                                psB[0][:, :],
                                wq_sb[:, d * HC * HD + 3 * HD: d * HC * HD + 4 * HD],
                                xts[(s, d)][:, :],
                                start=(d == 0), stop=(d == N_D - 1),
                            )
                            nc.tensor.matmul(
                                psB[1][:, :],
                                wk_sb[:, d * HD:(d + 1) * HD],
                                xts[(s, d)][:, :],
                                start=(d == 0), stop=(d == N_D - 1),
                            )
                            nc.tensor.matmul(
                                psB[2][:, :],
                                wv_sb[:, d * HD:(d + 1) * HD],
                                xts[(s, d)][:, :],
                                start=(d == 0), stop=(d == N_D - 1),
                            )
                        b_evicts = [
                            (psB[0],
                             qT[0:64, 3 * S + s * SB: 3 * S + (s + 1) * SB],
                             qT[64:128, 3 * S + s * SB: 3 * S + (s + 1) * SB]),
                            (psB[1],
                             kT[0:64, s * SB:(s + 1) * SB],
                             kT[64:128, s * SB:(s + 1) * SB]),
                        ]
                        if s == N_S - 1:
                            b_evicts.reverse()
                        for ps_, dr, di in b_evicts:
                            rope_evict(ps_, s, dr, di)
                        # v: bf16 copy to SBUF, PE-transpose 128-blocks into
                        # natural [k, hd] layout
                        vt = p1t.tile([PB, SB], BF16, tag="vt", bufs=2)
                        nc.scalar.copy(vt[:, :], psB[2][:, :])
                        for sb_i in range(SB // PB):
                            j = s * (SB // PB) + sb_i
                            pstv = p1ps.tile([PB, PB], BF16, tag="pstv", bufs=1)
                            nc.tensor.transpose(
                                pstv[:, :], vt[:, sb_i * PB:(sb_i + 1) * PB], ident[:, :]
                            )
                            nc.scalar.copy(
                                v_all[:, j * VBLK: j * VBLK + HD], pstv[:, :]
                            )
                        if s == 2:
                            early_chunk(2, 0)
                            early_chunk(3, 0)
                        elif s == 3:
                            early_chunk(2, 1)
                            early_chunk(3, 1)

                # ============== phases 2+3 (merged, pipelined) ==============
                with tc.tile_pool(name="attp", bufs=1) as attp:
                    wo_sb = attp.tile([PB, HC * D], BF16, tag="wo")
                    for h in range(HC):
                        nc.sync.dma_start(
                            out=wo_sb[:, h * D:(h + 1) * D],
                            in_=wo[h * PB:(h + 1) * PB, :],
                        )
                    with (
                        tc.tile_pool(name="p2t", bufs=1) as p2t,
                        tc.tile_pool(name="p2ps", bufs=1, space="PSUM") as p2ps,
                    ):
                        atts = {}

                        def emit_chunk2(h, j, c):
                            emit_chunk(h, j, c, p2t, p2ps, "pss", 3)

                        def chunks_for(i):
                            # lazy: chunk (h, j, c) is first needed by PV(i)
                            # at i = j + 4*c  (chunk covers 4 q-blocks)
                            cl = []
                            for c in range(4):
                                j = i - 4 * c
                                if 0 <= j < N_KB and c * ECHUNK < S - j * PB:
                                    cl.extend((h, j, c) for h in range(HC))
                            return cl

                        def emit_pv1(i, h):
                            psa = p2ps.tile([PB, VN], F32, tag="psa", bufs=2)
                            for j in range(i + 1):
                                lhsT = (
                                    ptds[(h, j)][:, :] if j == i
                                    else pts[(h, j)][:, (i - j) * PB:(i - j + 1) * PB]
                                )
                                nc.tensor.matmul(
                                    psa[:, :],
                                    lhsT,
                                    v_all[:, j * VBLK: j * VBLK + VN],
                                    start=(j == 0), stop=(j == i),
                                )
                            rinv = p2t.tile([PB, 1], F32, tag="rinv", bufs=4)
                            nc.vector.reciprocal(rinv[:, :], psa[:, HD:HD + 1])
                            attn = p2t.tile([PB, PB], BF16, tag="attn", bufs=4)
                            nc.scalar.mul(attn[:, :], psa[:, 0:HD], rinv[:, :])
                            pstT = p2ps.tile([PB, PB], BF16, tag="pstT", bufs=1)
                            nc.tensor.transpose(pstT[:, :], attn[:, :], ident[:, :])
                            at = p2t.tile([PB, PB], BF16, tag="attnT", bufs=12)
                            nc.vector.tensor_copy(at[:, :], pstT[:, :])
                            atts[(i, h)] = at

                        def emit_p3(i, split_dma=False):
                            ot = p2t.tile([PB, D], BF16, tag="ot", bufs=2)
                            for n0 in range(0, D, SB):
                                pso = p2ps.tile([PB, SB], F32, tag="pso", bufs=2)
                                for h in range(HC):
                                    nc.tensor.matmul(
                                        pso[:, :],
                                        atts[(i, h)][:, :],
                                        wo_sb[:, h * D + n0: h * D + n0 + SB],
                                        start=(h == 0), stop=(h == HC - 1),
                                    )
                                nc.vector.tensor_copy(ot[:, n0:n0 + SB], pso[:, :])
                                if split_dma:
                                    nc.sync.dma_start(
                                        out=out[i * PB:(i + 1) * PB, n0:n0 + SB],
                                        in_=ot[:, n0:n0 + SB],
                                    )
                            if not split_dma:
                                nc.sync.dma_start(out=out[i * PB:(i + 1) * PB, :], in_=ot[:, :])

                        # software pipeline: all chunks PV(i) reads are emitted
                        # before any PV(i, h); the NEXT step's chunks spread
                        # between this step's PV/p3 work so the in-order PE
                        # queue always holds dependency-ready work and the Act
                        # engine is fed at a steady rate.
                        for t in chunks_for(0) + chunks_for(1):
                            if (t[0], t[1]) in pts and t[2] == 0:
                                continue  # emitted during phase 1
                            emit_chunk2(*t)
                        for i in range(N_KB):
                            nxt = chunks_for(i + 2) if i + 2 < N_KB else []
                            nn = max(1, len(nxt))
                            emit_pv1(i, 0)
                            for t in nxt[: nn // 3]:
                                emit_chunk2(*t)
                            emit_pv1(i, 1)
                            if i >= 1:
                                emit_p3(i - 1)
                            for t in nxt[nn // 3: 2 * nn // 3]:
                                emit_chunk2(*t)
                            emit_pv1(i, 2)
                            for t in nxt[2 * nn // 3:]:
                                emit_chunk2(*t)
                            emit_pv1(i, 3)
                        emit_p3(N_KB - 1, split_dma=True)


def build_nc(repeat=1):
    nc = bacc.Bacc("TRN2", target_bir_lowering=False, debug=False, num_devices=N_CORES)
    io = {
        "xT": nc.dram_tensor("xT", [D, S], BF16, kind="ExternalInput"),
        "wq": nc.dram_tensor("wq", [D, HC * HD], BF16, kind="ExternalInput"),
        "wk": nc.dram_tensor("wk", [D, HD], BF16, kind="ExternalInput"),
        "wv": nc.dram_tensor("wv", [D, HD], BF16, kind="ExternalInput"),
        "wo": nc.dram_tensor("wo", [HC * HD, D], BF16, kind="ExternalInput"),
        "cosT": nc.dram_tensor("cosT", [HD // 2, S], F32, kind="ExternalInput"),
        "maskT": nc.dram_tensor("maskT", [PB, PB], BF16, kind="ExternalInput"),
        "sinT": nc.dram_tensor("sinT", [HD // 2, S], F32, kind="ExternalInput"),
        "out": nc.dram_tensor("out", [S, D], BF16, kind="ExternalOutput"),
    }
    with tile.TileContext(nc) as tc:
        emit_core_kernel(nc, tc, io, repeat=repeat)
    nc.compile()
    return nc


# ---------------------------------------------------------------------------
# host-side sharding + execution
# ---------------------------------------------------------------------------

_HALFSPLIT = np.concatenate([np.arange(0, HD, 2), np.arange(1, HD, 2)])


def _bf16():
    import ml_dtypes
    return ml_dtypes.bfloat16


def make_core_inputs(x, wq, wk, wv, wo, freqs_cos, freqs_sin):
    """Build the 8 per-core input dicts (numpy, host-side)."""
    bf16 = _bf16()
    scale = np.float32(1.0 / np.sqrt(HD))

    maskT = np.where(
        np.arange(PB)[None, :] >= np.arange(PB)[:, None], 1.0, 0.0
    ).astype(bf16)  # [k, q]: keep where q >= k
    xTs = [np.ascontiguousarray(x[b].T).astype(bf16) for b in range(B)]
    cosTs = [np.ascontiguousarray(freqs_cos[b].T).astype(np.float32) for b in range(B)]
    sinTs = [np.ascontiguousarray(freqs_sin[b].T).astype(np.float32) for b in range(B)]

    in_maps = []
    for c in range(N_CORES):
        b, g = divmod(c, N_KV_HEADS)
        qcols = np.concatenate([(HC * g + h) * HD + _HALFSPLIT for h in range(HC)])
        wq_c = (np.ascontiguousarray(wq[:, qcols]) * scale).astype(bf16)
        wk_c = np.ascontiguousarray(wk[:, g * HD + _HALFSPLIT]).astype(bf16)
        wv_c = np.ascontiguousarray(wv[:, g * HD:(g + 1) * HD]).astype(bf16)
        wo_c = np.ascontiguousarray(wo[g * HC * HD:(g + 1) * HC * HD, :]).astype(bf16)
        in_maps.append(
            {
                "xT": xTs[b],
                "wq": wq_c,
                "wk": wk_c,
                "wv": wv_c,
                "wo": wo_c,
                "cosT": cosTs[b],
                "maskT": maskT,
                "sinT": sinTs[b],
            }
        )
    return in_maps


_CACHE = {}


def get_runner(repeat=1, chain=1):
    """Build (once) the Bass module and a cached jitted 8-core executor."""
    if (repeat, chain) in _CACHE:
        return _CACHE[(repeat, chain)]
    import jax
    from jax.sharding import Mesh, PartitionSpec
    from jax.experimental.shard_map import shard_map
    from concourse.bass2jax import (
        _bass_exec_p,
        install_neuronx_cc_hook,
        partition_id_tensor,
    )

    nc = build_nc(repeat=repeat)
    install_neuronx_cc_hook()
    partition_name = nc.partition_id_tensor.name if nc.partition_id_tensor else None
    in_names, out_names, out_avals = [], [], []
    for alloc in nc.m.functions[0].allocations:
        if not isinstance(alloc, mybir.MemoryLocationSet):
            continue
        name = alloc.memorylocations[0].name
        if alloc.kind == "ExternalInput":
            if name != partition_name:
                in_names.append(name)
        elif alloc.kind == "ExternalOutput":
            out_names.append(name)
            out_avals.append(
                jax.core.ShapedArray(tuple(alloc.tensor_shape), mybir.dt.np(alloc.dtype))
            )
    n_params = len(in_names)
    n_outs = len(out_avals)
    all_in_names = list(in_names) + list(out_names)
    if partition_name is not None:
        all_in_names.append(partition_name)

    def _body(*args):
        operands = list(args)
        if partition_name is not None:
            operands.append(partition_id_tensor())
        outs = _bass_exec_p.bind(
            *operands,
            out_avals=tuple(out_avals),
            in_names=tuple(all_in_names),
            out_names=tuple(out_names),
            lowering_input_output_aliases=(),
            sim_require_finite=True,
            sim_require_nnan=True,
            nc=nc,
        )
        return tuple(outs)

    devices = jax.devices()[:N_CORES]
    mesh = Mesh(np.asarray(devices), ("core",))
    in_specs = (PartitionSpec("core"),) * (n_params + n_outs)
    out_specs = (PartitionSpec("core"),) * n_outs

    def _chain(*args):
        ins, outs = args[:n_params], args[n_params:]
        for _ in range(chain):
            outs = _body(*ins, *outs)
        return outs

    fn = jax.jit(
        shard_map(_chain, mesh=mesh, in_specs=in_specs, out_specs=out_specs, check_rep=False),
        keep_unused=True,
    )

    from jax.sharding import NamedSharding

    sh = NamedSharding(mesh, PartitionSpec("core"))

    def prepare(in_maps):
        concat_in = [
            np.concatenate([m[name] for m in in_maps], axis=0) for name in in_names
        ]
        concat_zeros = [
            np.zeros((N_CORES * a.shape[0], *a.shape[1:]), a.dtype) for a in out_avals
        ]
        return [jax.device_put(a, sh) for a in concat_in + concat_zeros]

    def run_dev(dev_args):
        out_arrs = fn(*dev_args)
        jax.block_until_ready(out_arrs)
        return out_arrs

    def run(in_maps):
        out_arrs = run_dev(prepare(in_maps))
        return np.asarray(out_arrs[0]).reshape(N_CORES, S, D)

    run.prepare = prepare
    run.run_dev = run_dev
    run.fn = fn
    _CACHE[(repeat, chain)] = run
    return run


def kernel(x, wq, wk, wv, wo, freqs_cos, freqs_sin):
    x = np.asarray(x, np.float32)
    wq = np.asarray(wq, np.float32)
    wk = np.asarray(wk, np.float32)
    wv = np.asarray(wv, np.float32)
    wo = np.asarray(wo, np.float32)
    freqs_cos = np.asarray(freqs_cos, np.float32)
    freqs_sin = np.asarray(freqs_sin, np.float32)

    in_maps = make_core_inputs(x, wq, wk, wv, wo, freqs_cos, freqs_sin)
    run = get_runner(repeat=1)
    partials = run(in_maps)  # [8, S, D] bf16
    partials = partials.astype(np.float32)
    out = np.stack(
        [partials[b * N_KV_HEADS:(b + 1) * N_KV_HEADS].sum(axis=0) for b in range(B)]
    )
    return out.astype(np.float32)
